# revision 23
# baseline (speedup 1.0000x reference)
"""Trainium2 Bass kernel for nn_Attention1D (B=4, L=4096, C=64).

reference:
    Q = x@Wq + bq ; K = x@Wk + bk ; V = x@Wv + bv          (per batch b)
    s = Q @ K.T / sqrt(C)                                   [L_q, L_k]
    attn = softmax(s, axis=q)      # normalize over QUERY axis
    out = attn @ V + x

Sharding: 8 cores = 4 batches x 2 key-shards (k in [0,2048) / [2048,4096)).
The softmax normalizes over q, which is NOT sharded, so each core's softmax
is fully local and the two k-shards' partial outputs simply ADD (host sums
the pair and adds the residual x).

Math folding: s = xq~ A~ xk~^T with A~ = [Wq;bq]@[Wk;bk]^T/sqrt(C) (65x65,
host-precomputed), so the Q projection disappears: score lhsT tiles are
KA = (A~ @ xk~^T)[0:64] and the rank-1 bias row (A~ xk~^T)[64,k] rides the
V projection as a 65th output column and enters exp() as a per-partition
bias (bias support stays fully general).

Layout: channel-major; scores come out transposed sT[k, q] with the softmax
axis on the free dim:
    sT chunk = matmul(lhsT=KA[0:64 / 64:128, k-tile], rhs=xqT-chunk) f32r,
               two k-tiles row-packed via tile_position (contract 64)
    exp      = [128,1024] PSUM chunks split across two engines:
               ACT (table exp, bias=b_s, accum_out -> Z partial) and
               DVE (Schraudolph: bf16 bits of e^s as the int16
               (s + b_s)*184.665 + 16250, one tensor_scalar mul-add with
               per-partition scalar2, int16 out aliased as bf16; Z partial
               from a DVE free-axis reduce of the bf16 view). GpSimd is
               ~20 G elem/s and cannot read PSUM -> only tiny SBUF ops.
    out      = PSUM-accumulated over 16 k-tiles:
               matmul(acc[qc], lhsT=ET[k, qc-chunk](bf16), rhs=GV[k, f])
GV = V * (1/Z) per k-tile, bf16. Q/K path runs in float32r. AV groups are
emitted BEFORE the score MMs of each chunk so the in-order PE queue can
fill score-slot waits with ready AV work.

PSUM (8 banks): 2 x [128,1024]f32 score slots (4 banks, double-buffered)
+ [128,32,64]f32 out accumulator (4 banks). matmul start=True clears
has_written (pending-zero) for the whole bank, so only the first
accumulator chunk-MM per bank sets it.

A ~7us dummy-matmul warmup burst runs during the input DMAs (PE p-state).
"""

import numpy as np
import ml_dtypes  # noqa: F401  (np bf16 support registered on import)

B, L, C = 4, 4096, 64
NCORES = 8
KSH = L // 2          # k columns per core: 2048
NKT = KSH // 128      # 16 k-tiles per core
NQC = L // 128        # 32 q-chunks of 128
NQ5 = L // 512        # 8 q-chunks of 512

LOG2E = 1.4426950408889634
SCH_A = LOG2E * 128.0            # Schraudolph scale to bf16 bits
SCH_B = 127.0 * 128.0 - 6.0      # bias - correction c=6 (tuned)

_cache = {}


def _build():
    import concourse.bacc as bacc
    import concourse.mybir as mybir
    import concourse.tile as tile
    from concourse.bass import _add_dep_helper

    bf16 = mybir.dt.bfloat16
    i16 = mybir.dt.int16
    f32 = mybir.dt.float32
    f32r = mybir.dt.float32r
    AF = mybir.ActivationFunctionType
    AX = mybir.AxisListType
    OP = mybir.AluOpType

    nc = bacc.Bacc("TRN2", target_bir_lowering=False, debug=False)

    xt_d = nc.dram_tensor("xt", [C, L], f32r, kind="ExternalInput")
    xk_d = nc.dram_tensor("xk", [C + 1, KSH], f32r, kind="ExternalInput")
    ka_d = nc.dram_tensor("ka", [C + 1, 2 * C], f32r, kind="ExternalInput")
    wv_d = nc.dram_tensor("wv", [C + 1, C + 2], f32r, kind="ExternalInput")
    o_d = nc.dram_tensor("o", [L, C], f32, kind="ExternalOutput")

    with tile.TileContext(nc) as tc:
        with (
            tc.tile_pool(name="consts", bufs=1) as consts,
            tc.tile_pool(name="sb", bufs=1) as sb,
            tc.tile_pool(name="etp", bufs=4) as etp,
            tc.tile_pool(name="gvp", bufs=4) as gvp,
            tc.tile_pool(name="zpp", bufs=6) as zpp,
            tc.tile_pool(name="scp", bufs=2, space="PSUM") as scp,
            tc.tile_pool(name="accp", bufs=1, space="PSUM") as accp,
        ):
            # --- PE warmup: dense dummy matmuls while the DMAs stream in ---
            wu = consts.tile([128, 512], bf16)
            nc.vector.memset(wu, 0.0)
            for _ in range(22):
                ps = scp.tile([128, 512], f32, tag="s")
                nc.tensor.matmul(ps, lhsT=wu[:, 0:128], rhs=wu,
                                 start=True, stop=True)

            ka_s = consts.tile([C + 1, 2 * C], f32r)
            wv_s = consts.tile([C + 1, C + 2], f32r)
            nc.sync.dma_start(out=ka_s, in_=ka_d.ap())
            nc.sync.dma_start(out=wv_s, in_=wv_d.ap())

            # xk first: it gates the KA/V projection chain; the xqT chunks
            # trickle in behind it (first score MM only needs chunk 0).
            xk_c = []
            for c in range(KSH // 512):
                t = sb.tile([C + 1, 512], f32r, tag=f"xk{c}")
                nc.sync.dma_start(out=t, in_=xk_d.ap()[:, c * 512:(c + 1) * 512])
                xk_c.append(t)
            # xqT chunks [128, 512]: rows 0-63 and 64-127 both hold xqT so
            # score matmuls can be row-packed two k-tiles at a time.
            xt_c = []
            for c in range(NQ5):
                t = sb.tile([128, 512], f32r, tag=f"xt{c}")
                src = xt_d.ap()[:, c * 512:(c + 1) * 512]
                nc.sync.dma_start(out=t[0:C, :], in_=src)
                nc.sync.dma_start(out=t[C:128, :], in_=src)
                xt_c.append(t)

            # KA tiles [128, 512]: rows 0-63 = (A~ xk~^T)[0:64], rows 64-127
            # duplicate (ka weights doubled host-side).
            kt_c = []
            for c in range(KSH // 512):
                ps = scp.tile([128, 512], f32, tag="s")
                nc.tensor.matmul(ps, lhsT=ka_s, rhs=xk_c[c],
                                 start=True, stop=True)
                t = sb.tile([128, 512], f32r, tag=f"kt{c}")
                nc.vector.tensor_copy(out=t, in_=ps)
                kt_c.append(t)

            v_ts = []   # V [k(128), f] per k-tile, bf16
            sb_ts = []  # score-bias per k-tile [128, 1] f32 (for ACT exp)
            sbb_ts = []  # Schraudolph scalar2 per k-tile [128, 1] f32
            for kt in range(NKT):
                vps = scp.tile([128, C + 2], f32, tag="s")
                nc.tensor.matmul(
                    vps,
                    lhsT=xk_c[kt // 4][:, (kt % 4) * 128:(kt % 4 + 1) * 128],
                    rhs=wv_s, start=True, stop=True,
                )
                v_t = sb.tile([128, C], bf16, tag=f"v{kt}")
                nc.vector.tensor_copy(out=v_t, in_=vps[:, 0:C])
                b_t = sb.tile([128, 1], f32, tag=f"sb{kt}")
                nc.vector.tensor_copy(out=b_t, in_=vps[:, C:C + 1])
                bb_t = sb.tile([128, 1], f32, tag=f"sbb{kt}")
                nc.gpsimd.tensor_scalar(out=bb_t, in0=b_t,
                                        scalar1=SCH_A, scalar2=SCH_B,
                                        op0=OP.mult, op1=OP.add)
                v_ts.append(v_t)
                sb_ts.append(b_t)
                sbb_ts.append(bb_t)

            # --- main loop over k-tile pairs ---
            acc = accp.tile([128, NQC, C], f32)   # 4 PSUM banks, whole loop
            prev = None

            # chunk -> engine: ACT 6x (exact exp + Z via accum) and DVE 2x
            ENG = {(0, 0): "act", (0, 1): "dve", (0, 2): "act", (0, 3): "act",
                   (1, 0): "act", (1, 1): "act", (1, 2): "dve", (1, 3): "act"}

            def emit_exp(ab, kt, st, et16, etbf, zp, c2):
                eng = ENG[(ab, c2)]
                if (ab, c2) in ((1, 1), (0, 2)):
                    # split chunk: ACT half + DVE half, so ACT's per-pair
                    # stream shortens and PSUM score slots free up sooner
                    q0 = c2 * 1024
                    nc.scalar.activation(
                        out=etbf[:, q0:q0 + 512], in_=st[:, 0:512],
                        func=AF.Exp, bias=sb_ts[kt],
                        accum_out=zp[:, 4:5],
                    )
                    nc.vector.tensor_scalar(
                        out=et16[:, q0 + 512:q0 + 1024], in0=st[:, 512:1024],
                        scalar1=SCH_A, scalar2=sbb_ts[kt],
                        op0=OP.mult, op1=OP.add,
                    )
                    nc.vector.tensor_reduce(
                        out=zp[:, c2:c2 + 1], in_=etbf[:, q0 + 512:q0 + 1024],
                        axis=AX.X, op=OP.add,
                    )
                elif eng == "act":
                    nc.scalar.activation(
                        out=etbf[:, c2 * 1024:(c2 + 1) * 1024], in_=st,
                        func=AF.Exp, bias=sb_ts[kt],
                        accum_out=zp[:, c2:c2 + 1],
                    )
                else:
                    nc.vector.tensor_scalar(
                        out=et16[:, c2 * 1024:(c2 + 1) * 1024], in0=st,
                        scalar1=SCH_A, scalar2=sbb_ts[kt],
                        op0=OP.mult, op1=OP.add,
                    )
                    nc.vector.tensor_reduce(
                        out=zp[:, c2:c2 + 1],
                        in_=etbf[:, c2 * 1024:(c2 + 1) * 1024],
                        axis=AX.X, op=OP.add,
                    )

            def emit_av_group(p, c2):
                # 8 AV chunk-MMs of the previous k-tile, interleaved between
                # score chunks to keep the PE dense.
                et_p, gv_p, kt_p = p
                for qc in range(c2 * 8, c2 * 8 + 8):
                    # start=True clears has_written for the WHOLE bank: only
                    # the first chunk-MM per bank may set it; later chunks
                    # overwrite-where-unset, which sets their own bits.
                    nc.tensor.matmul(
                        acc[:, qc, :],
                        lhsT=et_p[:, qc * 128:(qc + 1) * 128],
                        rhs=gv_p,
                        start=(kt_p == 0 and qc % 8 == 0),
                        stop=(kt_p == NKT - 1),
                        skip_group_check=True,
                    )

            for kp in range(NKT // 2):
                kA, kB = 2 * kp, 2 * kp + 1
                etA16 = etp.tile([128, L], i16, tag="etA")
                etB16 = etp.tile([128, L], i16, tag="etB")
                etA = etA16.bitcast(bf16)
                etB = etB16.bitcast(bf16)
                zpA = zpp.tile([128, 5], f32, tag="zpA")
                zpB = zpp.tile([128, 5], f32, tag="zpB")
                lA = kt_c[kA // 4][0:C, (kA % 4) * 128:(kA % 4 + 1) * 128]
                lB = kt_c[kB // 4][C:128, (kB % 4) * 128:(kB % 4 + 1) * 128]
                for c2 in range(4):
                    # AV of the previous pair FIRST: if the score MMs below
                    # stall on a PSUM slot (waiting for an exp), the in-order
                    # PE queue still has ready AV work in front of them.
                    if prev is not None:
                        emit_av_group(prev[0], c2)
                        emit_av_group(prev[1], c2)
                    stA = scp.tile([128, 1024], f32, tag="s")
                    stB = scp.tile([128, 1024], f32, tag="s")
                    last = None
                    for h in range(2):
                        rhs = xt_c[c2 * 2 + h]
                        ma = nc.tensor.matmul(
                            stA[:, h * 512:(h + 1) * 512], lhsT=lA,
                            rhs=rhs[0:C, :], tile_position=(0, 0),
                            start=True, stop=True,
                        )
                        mb = nc.tensor.matmul(
                            stB[:, h * 512:(h + 1) * 512], lhsT=lB,
                            rhs=rhs[C:128, :], tile_position=(C, 0),
                            start=True, stop=True,
                        )
                        # keep the A/B pair adjacent in the static PE order so
                        # the row-packed halves co-issue
                        if last is not None:
                            _add_dep_helper(ma.ins, last.ins, sync=False,
                                            reason="pair order")
                        _add_dep_helper(mb.ins, ma.ins, sync=False,
                                        reason="pair order")
                        last = mb
                    emit_exp(0, kA, stA, etA16, etA, zpA, c2)
                    emit_exp(1, kB, stB, etB16, etB, zpB, c2)
                gvs = []
                for kt, zp, vv in ((kA, zpA, v_ts[kA]), (kB, zpB, v_ts[kB])):
                    z = zpp.tile([128, 1], f32, tag=f"z{kt % 2}")
                    nc.vector.reduce_sum(out=z, in_=zp, axis=AX.X)
                    rz = zpp.tile([128, 1], f32, tag=f"rz{kt % 2}")
                    nc.vector.reciprocal(out=rz, in_=z)
                    gv = gvp.tile([128, C], bf16, tag=f"gv{kt % 2}")
                    nc.vector.tensor_scalar_mul(gv, vv, rz)
                    gvs.append(gv)
                prev = ((etA, gvs[0], kA), (etB, gvs[1], kB))
            # final pair's AV drain, interleaved with the per-bank
            # evacuation + store so the tail overlaps the remaining AV work
            o_ap = o_d.ap()
            for g in range(4):
                emit_av_group(prev[0], g)
                emit_av_group(prev[1], g)
                # finer pieces for the last bank so the final transfer is
                # small; round-robin over the DMA-capable engines' queues
                npc = 4 if g == 3 else 2
                for hh in range(npc):
                    w = 8 // npc
                    ob = sb.tile([128, w, C], f32, tag=f"ob{g}{hh}")
                    q0 = g * 8 + hh * w
                    nc.vector.tensor_copy(out=ob, in_=acc[:, q0:q0 + w, :])
                    eng = (nc.sync, nc.scalar, nc.gpsimd)[(g * 2 + hh) % 3]
                    eng.dma_start(
                        out=o_ap[q0 * 128:(q0 + w) * 128, :].rearrange(
                            "(t p) f -> p t f", p=128
                        ),
                        in_=ob,
                    )

    nc.compile()
    return nc


def _get_nc():
    if "nc" not in _cache:
        _cache["nc"] = _build()
    return _cache["nc"]


def _in_maps(x, Wq, bq, Wk, bk, Wv, bv):
    s = 1.0 / np.sqrt(np.float32(C))
    wq1 = np.concatenate([Wq, bq[None, :]], 0).astype(np.float32)   # [65,64]
    wk1 = np.concatenate([Wk, bk[None, :]], 0).astype(np.float32)   # [65,64]
    amat = (wq1 @ wk1.T) * s                                        # [65,65]
    # ka lhsT [65, 128]: cols 0-63 = A~^T[:, 0:64] (KA rows 0..63), doubled
    ka = np.concatenate([amat.T[:, 0:C], amat.T[:, 0:C]], 1).astype(np.float32)
    # V weights extended with the score-bias column A~[64, :] (+ f32r pad)
    wv1 = np.concatenate([Wv, bv[None, :]], 0).astype(np.float32)   # [65,64]
    wv1 = np.concatenate([wv1, amat[C:C + 1, :].T,
                          np.zeros((C + 1, 1), np.float32)], 1)      # [65,66]
    maps = []
    for core in range(NCORES):
        b, half = core // 2, core % 2
        xt = np.ascontiguousarray(x[b].T.astype(np.float32))        # [64, L]
        xk = np.ascontiguousarray(np.concatenate(
            [xt[:, half * KSH:(half + 1) * KSH],
             np.ones((1, KSH), np.float32)], 0
        ))                                                          # [65, KSH]
        maps.append({"xt": xt, "xk": xk, "ka": ka, "wv": wv1})
    return maps


def _unpack(outs, x):
    full = np.empty((B, L, C), np.float32)
    for b in range(B):
        full[b] = outs[2 * b] + outs[2 * b + 1] + x[b]
    return full


def _run(x, Wq, bq, Wk, bk, Wv, bv, trace=False):
    from concourse.bass_utils import run_bass_kernel_spmd

    nc = _get_nc()
    maps = _in_maps(x, Wq, bq, Wk, bk, Wv, bv)
    res = run_bass_kernel_spmd(
        nc, maps, core_ids=list(range(NCORES)), trace=trace
    )
    outs = [r["o"].astype(np.float32) for r in res.results]
    return _unpack(outs, x), res


def kernel(x, Wq, bq, Wk, bk, Wv, bv):
    x = np.asarray(x, np.float32)
    full, _ = _run(
        x,
        np.asarray(Wq, np.float32), np.asarray(bq, np.float32),
        np.asarray(Wk, np.float32), np.asarray(bk, np.float32),
        np.asarray(Wv, np.float32), np.asarray(bv, np.float32),
    )
    return full


# revision 24
# speedup vs baseline: 1.0665x; 1.0665x over previous
"""Trainium2 Bass kernel for nn_Attention1D (B=4, L=4096, C=64).

reference:
    Q = x@Wq + bq ; K = x@Wk + bk ; V = x@Wv + bv          (per batch b)
    s = Q @ K.T / sqrt(C)                                   [L_q, L_k]
    attn = softmax(s, axis=q)      # normalize over QUERY axis
    out = attn @ V + x

Sharding: 8 cores = 4 batches x 2 key-shards (k in [0,2048) / [2048,4096)).
The softmax normalizes over q, which is NOT sharded, so each core's softmax
is fully local and the two k-shards' partial outputs simply ADD (host sums
the pair and adds the residual x).

Math folding: s = xq~ A~ xk~^T with A~ = [Wq;bq]@[Wk;bk]^T/sqrt(C) (65x65,
host-precomputed), so the Q projection disappears: score lhsT tiles are
KA = (A~ @ xk~^T)[0:64] and the rank-1 bias row (A~ xk~^T)[64,k] rides the
V projection as a 65th output column and enters exp() as a per-partition
bias (bias support stays fully general).

Layout: channel-major; scores come out transposed sT[k, q] with the softmax
axis on the free dim:
    sT chunk = matmul(lhsT=KA[0:64 / 64:128, k-tile], rhs=xqT-chunk) f32r,
               two k-tiles row-packed via tile_position (contract 64)
    exp      = [128,1024] PSUM chunks split across two engines:
               ACT (table exp, bias=b_s, accum_out -> Z partial) and
               DVE (Schraudolph: bf16 bits of e^s as the int16
               (s + b_s)*184.665 + 16250, one tensor_scalar mul-add with
               per-partition scalar2, int16 out aliased as bf16; Z partial
               from a DVE free-axis reduce of the bf16 view). GpSimd is
               ~20 G elem/s and cannot read PSUM -> only tiny SBUF ops.
    out      = PSUM-accumulated over 16 k-tiles:
               matmul(acc[qc], lhsT=ET[k, qc-chunk](bf16), rhs=GV[k, f])
GV = V * (1/Z) per k-tile, bf16. Q/K path runs in float32r. AV groups are
emitted BEFORE the score MMs of each chunk so the in-order PE queue can
fill score-slot waits with ready AV work.

PSUM (8 banks): 2 x [128,1024]f32 score slots (4 banks, double-buffered)
+ [128,32,64]f32 out accumulator (4 banks). matmul start=True clears
has_written (pending-zero) for the whole bank, so only the first
accumulator chunk-MM per bank sets it.

A ~7us dummy-matmul warmup burst runs during the input DMAs (PE p-state).
"""

import numpy as np
import ml_dtypes  # noqa: F401  (np bf16 support registered on import)

B, L, C = 4, 4096, 64
NCORES = 8
KSH = L // 2          # k columns per core: 2048
NKT = KSH // 128      # 16 k-tiles per core
NQC = L // 128        # 32 q-chunks of 128
NQ5 = L // 512        # 8 q-chunks of 512

LOG2E = 1.4426950408889634
SCH_A = LOG2E * 128.0            # Schraudolph scale to bf16 bits
SCH_B = 127.0 * 128.0 - 6.0      # bias - correction c=6 (tuned)

_cache = {}


def _build():
    import concourse.bacc as bacc
    import concourse.mybir as mybir
    import concourse.tile as tile
    from concourse.bass import _add_dep_helper

    bf16 = mybir.dt.bfloat16
    i16 = mybir.dt.int16
    f32 = mybir.dt.float32
    f32r = mybir.dt.float32r
    AF = mybir.ActivationFunctionType
    AX = mybir.AxisListType
    OP = mybir.AluOpType

    nc = bacc.Bacc("TRN2", target_bir_lowering=False, debug=False)

    xt_d = nc.dram_tensor("xt", [C, L], f32r, kind="ExternalInput")
    xk_d = nc.dram_tensor("xk", [C + 1, KSH], f32r, kind="ExternalInput")
    ka_d = nc.dram_tensor("ka", [C + 1, 2 * C], f32r, kind="ExternalInput")
    wv_d = nc.dram_tensor("wv", [C + 1, C + 2], f32r, kind="ExternalInput")
    o_d = nc.dram_tensor("o", [L, C], f32, kind="ExternalOutput")

    with tile.TileContext(nc) as tc:
        with (
            tc.tile_pool(name="consts", bufs=1) as consts,
            tc.tile_pool(name="sb", bufs=1) as sb,
            tc.tile_pool(name="etp", bufs=4) as etp,
            tc.tile_pool(name="gvp", bufs=4) as gvp,
            tc.tile_pool(name="zpp", bufs=6) as zpp,
            tc.tile_pool(name="scp", bufs=2, space="PSUM") as scp,
            tc.tile_pool(name="accp", bufs=1, space="PSUM") as accp,
        ):
            # --- PE warmup: dense dummy matmuls while the DMAs stream in ---
            wu = consts.tile([128, 512], bf16)
            nc.vector.memset(wu, 0.0)
            for _ in range(14):
                ps = scp.tile([128, 512], f32, tag="s")
                nc.tensor.matmul(ps, lhsT=wu[:, 0:128], rhs=wu,
                                 start=True, stop=True)

            ka_s = consts.tile([C + 1, 2 * C], f32r)
            wv_s = consts.tile([C + 1, C + 2], f32r)
            nc.sync.dma_start(out=ka_s, in_=ka_d.ap())
            nc.sync.dma_start(out=wv_s, in_=wv_d.ap())

            # xk first: it gates the KA/V projection chain; the xqT chunks
            # trickle in behind it (first score MM only needs chunk 0).
            xk_c = []
            for c in range(KSH // 512):
                t = sb.tile([C + 1, 512], f32r, tag=f"xk{c}")
                nc.sync.dma_start(out=t, in_=xk_d.ap()[:, c * 512:(c + 1) * 512])
                xk_c.append(t)
            # xqT chunks [128, 512]: rows 0-63 and 64-127 both hold xqT so
            # score matmuls can be row-packed two k-tiles at a time.
            xt_c = []
            for c in range(NQ5):
                t = sb.tile([128, 512], f32r, tag=f"xt{c}")
                src = xt_d.ap()[:, c * 512:(c + 1) * 512]
                nc.sync.dma_start(out=t[0:C, :], in_=src)
                nc.sync.dma_start(out=t[C:128, :], in_=src)
                xt_c.append(t)

            # KA tiles [128, 512]: rows 0-63 = (A~ xk~^T)[0:64], rows 64-127
            # duplicate (ka weights doubled host-side).
            kt_c = []
            for c in range(KSH // 512):
                ps = scp.tile([128, 512], f32, tag="s")
                nc.tensor.matmul(ps, lhsT=ka_s, rhs=xk_c[c],
                                 start=True, stop=True)
                t = sb.tile([128, 512], f32r, tag=f"kt{c}")
                nc.vector.tensor_copy(out=t, in_=ps)
                kt_c.append(t)

            v_ts = []   # V [k(128), f] per k-tile, bf16
            sb_ts = []  # score-bias per k-tile [128, 1] f32 (for ACT exp)
            sbb_ts = []  # Schraudolph scalar2 per k-tile [128, 1] f32
            for kt in range(NKT):
                vps = scp.tile([128, C + 2], f32, tag="s")
                nc.tensor.matmul(
                    vps,
                    lhsT=xk_c[kt // 4][:, (kt % 4) * 128:(kt % 4 + 1) * 128],
                    rhs=wv_s, start=True, stop=True,
                )
                v_t = sb.tile([128, C], bf16, tag=f"v{kt}")
                nc.vector.tensor_copy(out=v_t, in_=vps[:, 0:C])
                b_t = sb.tile([128, 1], f32, tag=f"sb{kt}")
                nc.vector.tensor_copy(out=b_t, in_=vps[:, C:C + 1])
                bb_t = sb.tile([128, 1], f32, tag=f"sbb{kt}")
                nc.gpsimd.tensor_scalar(out=bb_t, in0=b_t,
                                        scalar1=SCH_A, scalar2=SCH_B,
                                        op0=OP.mult, op1=OP.add)
                v_ts.append(v_t)
                sb_ts.append(b_t)
                sbb_ts.append(bb_t)

            # --- main loop over k-tile pairs ---
            acc = accp.tile([128, NQC, C], f32)   # 4 PSUM banks, whole loop
            prev = None

            # chunk -> engine: ACT 6x (exact exp + Z via accum) and DVE 2x
            ENG = {(0, 0): "act", (0, 1): "dve", (0, 2): "act", (0, 3): "act",
                   (1, 0): "act", (1, 1): "act", (1, 2): "dve", (1, 3): "act"}

            def emit_exp(ab, kt, st, et16, etbf, zp, c2):
                eng = ENG[(ab, c2)]
                if (ab, c2) in ((1, 1),):
                    # split chunk: ACT half + DVE half, so ACT's per-pair
                    # stream shortens and PSUM score slots free up sooner
                    q0 = c2 * 1024
                    nc.scalar.activation(
                        out=etbf[:, q0:q0 + 512], in_=st[:, 0:512],
                        func=AF.Exp, bias=sb_ts[kt],
                        accum_out=zp[:, 4:5],
                    )
                    nc.vector.tensor_scalar(
                        out=et16[:, q0 + 512:q0 + 1024], in0=st[:, 512:1024],
                        scalar1=SCH_A, scalar2=sbb_ts[kt],
                        op0=OP.mult, op1=OP.add,
                    )
                    nc.vector.tensor_reduce(
                        out=zp[:, c2:c2 + 1], in_=etbf[:, q0 + 512:q0 + 1024],
                        axis=AX.X, op=OP.add,
                    )
                elif eng == "act":
                    nc.scalar.activation(
                        out=etbf[:, c2 * 1024:(c2 + 1) * 1024], in_=st,
                        func=AF.Exp, bias=sb_ts[kt],
                        accum_out=zp[:, c2:c2 + 1],
                    )
                else:
                    nc.vector.tensor_scalar(
                        out=et16[:, c2 * 1024:(c2 + 1) * 1024], in0=st,
                        scalar1=SCH_A, scalar2=sbb_ts[kt],
                        op0=OP.mult, op1=OP.add,
                    )
                    nc.vector.tensor_reduce(
                        out=zp[:, c2:c2 + 1],
                        in_=etbf[:, c2 * 1024:(c2 + 1) * 1024],
                        axis=AX.X, op=OP.add,
                    )

            def emit_av_group(p, c2):
                # 8 AV chunk-MMs of the previous k-tile, interleaved between
                # score chunks to keep the PE dense.
                et_p, gv_p, kt_p = p
                for qc in range(c2 * 8, c2 * 8 + 8):
                    # start=True clears has_written for the WHOLE bank: only
                    # the first chunk-MM per bank may set it; later chunks
                    # overwrite-where-unset, which sets their own bits.
                    nc.tensor.matmul(
                        acc[:, qc, :],
                        lhsT=et_p[:, qc * 128:(qc + 1) * 128],
                        rhs=gv_p,
                        start=(kt_p == 0 and qc % 8 == 0),
                        stop=(kt_p == NKT - 1),
                        skip_group_check=True,
                    )

            for kp in range(NKT // 2):
                kA, kB = 2 * kp, 2 * kp + 1
                etA16 = etp.tile([128, L], i16, tag="etA")
                etB16 = etp.tile([128, L], i16, tag="etB")
                etA = etA16.bitcast(bf16)
                etB = etB16.bitcast(bf16)
                zpA = zpp.tile([128, 5], f32, tag="zpA")
                zpB = zpp.tile([128, 5], f32, tag="zpB")
                lA = kt_c[kA // 4][0:C, (kA % 4) * 128:(kA % 4 + 1) * 128]
                lB = kt_c[kB // 4][C:128, (kB % 4) * 128:(kB % 4 + 1) * 128]
                for c2 in range(4):
                    # AV of the previous pair FIRST: if the score MMs below
                    # stall on a PSUM slot (waiting for an exp), the in-order
                    # PE queue still has ready AV work in front of them.
                    if prev is not None:
                        emit_av_group(prev[0], c2)
                        emit_av_group(prev[1], c2)
                    stA = scp.tile([128, 1024], f32, tag="s")
                    stB = scp.tile([128, 1024], f32, tag="s")
                    last = None
                    for h in range(2):
                        rhs = xt_c[c2 * 2 + h]
                        ma = nc.tensor.matmul(
                            stA[:, h * 512:(h + 1) * 512], lhsT=lA,
                            rhs=rhs[0:C, :], tile_position=(0, 0),
                            start=True, stop=True,
                        )
                        mb = nc.tensor.matmul(
                            stB[:, h * 512:(h + 1) * 512], lhsT=lB,
                            rhs=rhs[C:128, :], tile_position=(C, 0),
                            start=True, stop=True,
                        )
                        # keep the A/B pair adjacent in the static PE order so
                        # the row-packed halves co-issue
                        if last is not None:
                            _add_dep_helper(ma.ins, last.ins, sync=False,
                                            reason="pair order")
                        _add_dep_helper(mb.ins, ma.ins, sync=False,
                                        reason="pair order")
                        last = mb
                    emit_exp(0, kA, stA, etA16, etA, zpA, c2)
                    emit_exp(1, kB, stB, etB16, etB, zpB, c2)
                gvs = []
                for kt, zp, vv in ((kA, zpA, v_ts[kA]), (kB, zpB, v_ts[kB])):
                    z = zpp.tile([128, 1], f32, tag=f"z{kt % 2}")
                    nc.vector.reduce_sum(out=z, in_=zp, axis=AX.X)
                    rz = zpp.tile([128, 1], f32, tag=f"rz{kt % 2}")
                    nc.vector.reciprocal(out=rz, in_=z)
                    gv = gvp.tile([128, C], bf16, tag=f"gv{kt % 2}")
                    nc.vector.tensor_scalar_mul(gv, vv, rz)
                    gvs.append(gv)
                prev = ((etA, gvs[0], kA), (etB, gvs[1], kB))
            # final pair's AV drain, interleaved with the per-bank
            # evacuation + store so the tail overlaps the remaining AV work
            o_ap = o_d.ap()
            for g in range(4):
                emit_av_group(prev[0], g)
                emit_av_group(prev[1], g)
                # finer pieces for the last bank so the final transfer is
                # small; round-robin over the DMA-capable engines' queues
                npc = 4 if g == 3 else 2
                for hh in range(npc):
                    w = 8 // npc
                    ob = sb.tile([128, w, C], f32, tag=f"ob{g}{hh}")
                    q0 = g * 8 + hh * w
                    nc.vector.tensor_copy(out=ob, in_=acc[:, q0:q0 + w, :])
                    eng = (nc.sync, nc.scalar, nc.gpsimd)[(g * 2 + hh) % 3]
                    eng.dma_start(
                        out=o_ap[q0 * 128:(q0 + w) * 128, :].rearrange(
                            "(t p) f -> p t f", p=128
                        ),
                        in_=ob,
                    )

    nc.compile()
    return nc


def _get_nc():
    if "nc" not in _cache:
        _cache["nc"] = _build()
    return _cache["nc"]


def _in_maps(x, Wq, bq, Wk, bk, Wv, bv):
    s = 1.0 / np.sqrt(np.float32(C))
    wq1 = np.concatenate([Wq, bq[None, :]], 0).astype(np.float32)   # [65,64]
    wk1 = np.concatenate([Wk, bk[None, :]], 0).astype(np.float32)   # [65,64]
    amat = (wq1 @ wk1.T) * s                                        # [65,65]
    # ka lhsT [65, 128]: cols 0-63 = A~^T[:, 0:64] (KA rows 0..63), doubled
    ka = np.concatenate([amat.T[:, 0:C], amat.T[:, 0:C]], 1).astype(np.float32)
    # V weights extended with the score-bias column A~[64, :] (+ f32r pad)
    wv1 = np.concatenate([Wv, bv[None, :]], 0).astype(np.float32)   # [65,64]
    wv1 = np.concatenate([wv1, amat[C:C + 1, :].T,
                          np.zeros((C + 1, 1), np.float32)], 1)      # [65,66]
    maps = []
    for core in range(NCORES):
        b, half = core // 2, core % 2
        xt = np.ascontiguousarray(x[b].T.astype(np.float32))        # [64, L]
        xk = np.ascontiguousarray(np.concatenate(
            [xt[:, half * KSH:(half + 1) * KSH],
             np.ones((1, KSH), np.float32)], 0
        ))                                                          # [65, KSH]
        maps.append({"xt": xt, "xk": xk, "ka": ka, "wv": wv1})
    return maps


def _unpack(outs, x):
    full = np.empty((B, L, C), np.float32)
    for b in range(B):
        full[b] = outs[2 * b] + outs[2 * b + 1] + x[b]
    return full


def _run(x, Wq, bq, Wk, bk, Wv, bv, trace=False):
    from concourse.bass_utils import run_bass_kernel_spmd

    nc = _get_nc()
    maps = _in_maps(x, Wq, bq, Wk, bk, Wv, bv)
    res = run_bass_kernel_spmd(
        nc, maps, core_ids=list(range(NCORES)), trace=trace
    )
    outs = [r["o"].astype(np.float32) for r in res.results]
    return _unpack(outs, x), res


def kernel(x, Wq, bq, Wk, bk, Wv, bv):
    x = np.asarray(x, np.float32)
    full, _ = _run(
        x,
        np.asarray(Wq, np.float32), np.asarray(bq, np.float32),
        np.asarray(Wk, np.float32), np.asarray(bk, np.float32),
        np.asarray(Wv, np.float32), np.asarray(bv, np.float32),
    )
    return full
